# revision 2
# baseline (speedup 1.0000x reference)
"""Trainium2 Bass kernel: multi-head causal self-attention with RoPE.

Computes, for x:[B,S,D], Wq/Wk/Wv/Wo:[D,D] (B=2, S=2048, D=1024, H=16 heads,
hd=64):
    q/k/v = (x @ W{q,k,v}.T) -> [B,H,S,hd];  q,k = rope(q), rope(k)
    out   = softmax(causal(q k^T / sqrt(hd))) v   -> merge heads -> @ Wo.T

Sharding: 8 NeuronCores = (2 batches) x (4 head-groups of 4 heads).  Each
core computes its 4 heads' attention plus the partial output projection
(columns of Wo belonging to its heads); the host sums the 4 partial outputs
per batch.

Per-core dataflow (everything in "transposed" space so no PE transposes are
needed):
    xT [D,S] -> QT,KT [hd,S] per head (fp32r matmuls) -> RoPE (DVE shuffle
    + mul/add) -> scoresT[k,q] = KT^T-slice matmuls -> exp on ACT (no
    max-subtraction: |scores/8| <= ~3.2) -> PV with a ones-column appended
    to V so row 64 of the accumulator is the softmax denominator ->
    normalize -> output projection from the transposed head outputs.
"""

import sys

sys.path.insert(0, "/opt/trn_rl_repo")

import numpy as np

import concourse.bass as bass
import concourse.mybir as mybir
import concourse.tile as tile
from concourse.bass_utils import run_bass_kernel_spmd

F32 = mybir.dt.float32
F32R = mybir.dt.float32r
BF16 = mybir.dt.bfloat16
AF = mybir.ActivationFunctionType
OP = mybir.AluOpType

# stream_shuffle's 32-entry mask is a per-quadrant partition permutation
# (applied identically to all four 32-partition quadrants).  We therefore
# store head dims interleaved -- partition 64h+2i holds dim i, 64h+2i+1
# holds dim 32+i -- so the RoPE pair swap is an adjacent-pair exchange.
# The interleave is a shared permutation of Q and K dims (folded into the
# weight slices and rope tables on the host), which leaves q.k scores
# unchanged.
SWAP_MASK = [i ^ 1 for i in range(32)]

HD = 64
HALF = HD // 2
ROPE_BASE = 10000.0
EVAC_ON_ACT = False   # A/B: route half the oproj evac copies to ACT


def _split_waits(nc, maxw=1):
    """walrus in this container rejects instructions with more than a couple
    of semaphore waits; hoist excess waits onto preceding NoOps."""
    ctr = 0
    for bb in nc.main_func.blocks:
        insts = bb.instructions
        new = []
        changed = False
        for ins in insts:
            si = ins.sync_info
            if si is not None and si.on_wait and len(si.on_wait) > maxw:
                waits = list(si.on_wait)
                keep, rest = waits[:maxw], waits[maxw:]
                for i in range(0, len(rest), maxw):
                    ctr += 1
                    new.append(mybir.InstNoOp(
                        name=f"WSPLIT-{ctr}", opcode="NoOp", engine=ins.engine,
                        sync_info=mybir.SyncInfo(on_wait=rest[i:i + maxw], on_update=[])))
                si.on_wait = keep
                changed = True
            new.append(ins)
        if changed:
            bb.instructions = new


def build_program(S, D, HPC=4, repeat=1, use_loop=False, phase=4):
    """One-core SPMD program: attention for HPC heads of one batch.

    phase (for perf bisection): -1 = input DMA only, 0 = +QKV matmuls,
    1 = +rope/V-evac, 2 = +scores/exp/PV, 3 = +normalize, 4 = full.
    use_loop wraps `repeat` copies of the body in a tc.For_i (constant
    NEFF size -- used by the loop-slope timing harness).
    """
    NKT = D // 128          # k-tiles over the embedding dim
    NSC = S // 512          # 512-wide s-chunks
    NST = S // 128          # 128-wide s-tiles
    G = HPC // 2            # head pairs
    E = HPC * HD            # per-core head dims

    nc = bass.Bass()
    # pre-tiled host layouts: xT[p, sc*NKT*512 + kt*512 + s'] = x[sc*512+s', kt*128+p]
    xT = nc.declare_dram_parameter("xT", [128, S * NKT], BF16, isOutput=False)
    wq = nc.declare_dram_parameter("wq", [128, NKT * E], BF16, isOutput=False)
    wk = nc.declare_dram_parameter("wk", [128, NKT * E], BF16, isOutput=False)
    wv = nc.declare_dram_parameter("wv", [128, NKT * E], BF16, isOutput=False)
    wo = nc.declare_dram_parameter("wo", [128, G * D], BF16, isOutput=False)
    cs = nc.declare_dram_parameter("cs", [128, S], BF16, isOutput=False)
    sn = nc.declare_dram_parameter("sn", [128, S], BF16, isOutput=False)
    tri = nc.declare_dram_parameter("tri", [128, 128], BF16, isOutput=False)
    idn = nc.declare_dram_parameter("idn", [128, 128], BF16, isOutput=False)
    out = nc.declare_dram_parameter("out", [S, D], BF16, isOutput=True)

    with tile.TileContext(nc) as tc, \
         nc.allow_low_precision(reason="float32r operands feed the PE at full rate"):
        with (tc.tile_pool(name="wp", bufs=1) as wp,
              tc.tile_pool(name="xp", bufs=2) as xp,
              tc.tile_pool(name="rt", bufs=3) as rt,
              tc.tile_pool(name="ptp", bufs=4) as ptp,
              tc.tile_pool(name="rcp", bufs=3) as rcp,
              tc.tile_pool(name="oevp", bufs=3) as oevp,
              tc.tile_pool(name="ps", bufs=1, space="PSUM") as ps_pool,
              tc.tile_pool(name="ot_ps", bufs=2, space="PSUM") as ot_ps):
            qkv_ps = mm_ps = ps_pool  # shared PSUM pool; tags set per tile

            # ---- persistent tiles
            wq_s = wp.tile([128, NKT * E], BF16, name="wq_s")
            wk_s = wp.tile([128, NKT * E], BF16, name="wk_s")
            wv_s = wp.tile([128, NKT * E], BF16, name="wv_s")
            wo_s = wp.tile([128, G * D], BF16, name="wo_s")
            cs_s = wp.tile([128, S], BF16, name="cs_s")
            sn_s = wp.tile([128, S], BF16, name="sn_s")
            tri_s = wp.tile([128, 128], BF16, name="tri_s")
            sel_s = wp.tile([128, 128], F32R, name="sel_s")
            idn_s = wp.tile([128, 128], BF16, name="idn_s")
            qtr = wp.tile([128, G * S], BF16, name="qtr")
            ktr = wp.tile([128, G * S], BF16, name="ktr")
            vv = wp.tile([128, NST * HPC * (HD + 1)], BF16, name="vv")
            # per-q-chunk otn tiles: oproj(qc-1) must not false-depend on
            # the (qc) normalize writes, which one big tile would impose
            otn_q = [wp.tile([128, G * 512], BF16, name=f"otn{i}")
                     for i in range(S // 512)]

            # weight/table loads issue from the ACT queue so the first x
            # chunk (SP queue) isn't stuck behind them; order = first use
            nc.scalar.dma_start(wq_s[:], wq[:])
            nc.scalar.dma_start(wk_s[:], wk[:])
            nc.scalar.dma_start(cs_s[:], cs[:])
            nc.scalar.dma_start(sn_s[:], sn[:])
            nc.scalar.dma_start(wv_s[:], wv[:])
            nc.scalar.dma_start(tri_s[:], tri[:])
            nc.scalar.dma_start(idn_s[:], idn[:])
            nc.scalar.dma_start(wo_s[:], wo[:])
            # ones columns of V+ (for the softmax denominator)
            vv_r = vv[:].rearrange("p (st h c) -> p st h c", st=NST, h=HPC, c=HD + 1)
            nc.vector.memset(vv_r[:, :, :, HD:HD + 1].bitcast(mybir.dt.uint16), 0x3f80)
            # sel: broadcast rows for the merged normalize -- row 64 selects
            # out-partitions 0:64 (head A), row 96 selects 64:128 (head B);
            # quadrant-aligned starts (engine partition accesses must begin
            # at 0/32/64/96)
            nc.vector.memset(sel_s[HD:HD + 1, :].bitcast(F32), 0.0)
            nc.vector.memset(sel_s[32:33, :].bitcast(F32), 0.0)
            nc.vector.memset(sel_s[HD:HD + 1, 0:HD].bitcast(F32), 1.0)
            nc.vector.memset(sel_s[32:33, HD:128].bitcast(F32), 1.0)

            def body():
                pending_norm = []
                # ================= QKV + RoPE =================
                def qkv_dma(sc):
                    xta = xp.tile([128, NKT * 512], BF16, tag="x", name="xta")
                    nc.sync.dma_start(xta[:], xT[:, sc * NKT * 512:(sc + 1) * NKT * 512])
                    return xta

                def qkv_chunk(sc, xta):
                    xts = [xta[:, kt * 512:(kt + 1) * 512] for kt in range(NKT)]

                    if phase == -1:
                        return
                    # Q^T and K^T, one head-pair (128 dims) at a time
                    for w_s, dst in ((wq_s, qtr), (wk_s, ktr)):
                        for g in range(G):
                            ps = qkv_ps.tile([128, 512], F32, tag="mm", bufs=2, name="ps")
                            for kt in range(NKT):
                                nc.tensor.matmul(
                                    ps[:], w_s[:, kt * E + g * 128: kt * E + (g + 1) * 128],
                                    xts[kt], start=(kt == 0), stop=(kt == NKT - 1))
                            if phase == 0:
                                continue
                            # RoPE: rot = ps*cos + swap(ps)*sgn_sin
                            qsw = rt.tile([128, 512], F32, tag="qsw", name="qsw")
                            m1 = rt.tile([128, 512], F32, tag="m1", name="m1")
                            m2 = rt.tile([128, 512], F32, tag="m2", name="m2")
                            # DVE does only the PSUM-reading shuffle + the m1
                            # product; the SBUF-only half of RoPE runs on the
                            # otherwise-idle Pool engine
                            nc.vector.stream_shuffle(qsw[:], ps[:], SWAP_MASK)
                            nc.vector.tensor_tensor(m1[:], ps[:], cs_s[:, sc * 512:(sc + 1) * 512], OP.mult)
                            nc.gpsimd.tensor_tensor(m2[:], qsw[:], sn_s[:, sc * 512:(sc + 1) * 512], OP.mult)
                            nc.gpsimd.tensor_tensor(
                                dst[:, g * S + sc * 512: g * S + (sc + 1) * 512], m1[:], m2[:], OP.add)

                    # V (natural layout) for the 4 s-tiles of this chunk
                    for stl in range(4):
                        st = sc * 4 + stl
                        ps = qkv_ps.tile([128, 512], F32, tag="mm", bufs=2, name="psv")
                        for kt in range(NKT):
                            nc.tensor.matmul(
                                ps[:, 0:E], xta[:, kt * 512 + stl * 128: kt * 512 + (stl + 1) * 128],
                                wv_s[:, kt * E:(kt + 1) * E], start=(kt == 0), stop=(kt == NKT - 1))
                        if phase == 0:
                            continue
                        dst = vv_r[:, st, :, 0:HD]
                        nc.vector.tensor_copy(dst, ps[:, 0:E].rearrange("p (h c) -> p h c", h=HPC, c=HD))

                # ================= attention + output projection =================
                # Per (q-chunk, head-pair): the two heads' K=64 score matmuls
                # sit in different PE row groups (partition bases 0/64) and
                # overlap in the array.  k-tiles are processed two at a time
                # so one exp covers a [128,1024] two-bank PSUM tile.  The
                # causal mask is ADDED into the scores by an identity matmul
                # of a 0/-400 constant (exp then yields exact-enough zeros).
                VW = HD + 1

                def oproj_pieces(qc, tail=False):
                    """oproj(qc) as 16 closures of ~2 matmuls + a copy each,
                    for interleaving into the next q-chunk's attention loop
                    (PE cover for the exp hops; hides the mm->evac ping-pong)."""
                    CW = min(512, D)
                    oev_box = {}
                    pieces = []

                    def piece(st2, half, st, nch, on_act=False):
                        if half == 0 and nch == 0:
                            oev_box[st2] = oevp.tile([128, 2 * D], BF16, tag="oev", name="oev")
                        oev = oev_box[st2]
                        op = mm_ps.tile([128, 512], F32, tag="mm", bufs=2, name="opps")
                        for g in range(G):
                            nc.tensor.matmul(
                                op[:, 0:CW],
                                otn_q[qc][:, g * 512 + (st - qc * 4) * 128:
                                          g * 512 + (st - qc * 4 + 1) * 128],
                                wo_s[:, g * D + nch * CW: g * D + (nch + 1) * CW],
                                start=(g == 0), stop=(g == G - 1))
                        # in the tail block every exp is done, so ACT is idle:
                        # alternating the evac copies halves the serial drain
                        if on_act:
                            nc.scalar.copy(
                                oev[:, half * D + nch * CW: half * D + (nch + 1) * CW], op[:, 0:CW])
                        else:
                            nc.vector.tensor_copy(
                                oev[:, half * D + nch * CW: half * D + (nch + 1) * CW], op[:, 0:CW])
                        if half == 1 and nch == D // CW - 1:
                            st0 = qc * 4 + st2 * 2
                            dst = out[st0 * 128:(st0 + 2) * 128, :].rearrange(
                                "(b p) d -> p b d", b=2)
                            nc.sync.dma_start(dst, oev[:].rearrange("p (b d) -> p b d", b=2))

                    idx = 0
                    for st2 in range(2):
                        for half in range(2):
                            for nch in range(D // CW):
                                on_act = tail and idx % 2 == 1
                                pieces.append(
                                    lambda st2=st2, half=half, nch=nch, on_act=on_act:
                                    piece(st2, half, qc * 4 + st2 * 2 + half, nch, on_act))
                                idx += 1
                    return pieces

                def flush_norm():
                    while pending_norm:
                        pending_norm.pop(0)()

                def attention_qc(qc, fillers=()):
                    fillers = list(fillers)
                    for g in range(G):
                        h0, h1 = 2 * g, 2 * g + 1
                        otA = ot_ps.tile([128, 512], F32, tag="ot", name="otA")
                        otB = ot_ps.tile([128, 512], F32, tag="ot", name="otB")
                        nkt_q = 4 * qc + 4

                        def emit_pv(kt, pt):
                            coff = max(0, 128 * kt - 512 * qc)
                            for hh, cb, ot in ((h0, 0, otA), (h1, 512, otB)):
                                nc.tensor.matmul(
                                    ot[0:VW, coff:512],
                                    vv[:, kt * HPC * VW + hh * VW: kt * HPC * VW + (hh + 1) * VW],
                                    pt[:, cb + coff:cb + 512],
                                    start=(kt == 0), stop=(kt == nkt_q - 1))

                        # software-pipelined: PV(kt-1) is emitted between the
                        # scores matmuls of kt and the exp of kt, so the PE
                        # never sits waiting on the exp it just enabled.  Both
                        # heads' scores live in one 2-bank PSUM tile (A at
                        # cols 0:512, B at 512:1024) so a single exp with a
                        # [p, 2, 512-coff] access pattern covers the pair.
                        pend = None
                        for kt in range(nkt_q):
                            coff = max(0, 128 * kt - 512 * qc)
                            diag = kt >= 4 * qc
                            s2 = mm_ps.tile([128, 1024], F32, tag="sc", bufs=2, name="s2")
                            for hp, cb in ((0, 0), (64, 512)):
                                nc.tensor.matmul(
                                    s2[:, cb + coff:cb + 512],
                                    ktr[hp:hp + 64, g * S + kt * 128: g * S + (kt + 1) * 128],
                                    qtr[hp:hp + 64, g * S + qc * 512 + coff: g * S + (qc + 1) * 512],
                                    start=True, stop=not diag)
                                if diag:
                                    nc.tensor.matmul(
                                        s2[:, cb + coff:cb + coff + 128],
                                        idn_s[:], tri_s[:], start=False, stop=True)
                            if kt == 0:
                                flush_norm()
                            if pend is not None:
                                emit_pv(*pend)
                            if len(fillers) > 3:
                                fillers.pop(0)()
                            pt = ptp.tile([128, 1024], BF16, tag="pt", bufs=4, name="pt")
                            sv = s2[:].rearrange("p (h c) -> p h c", h=2, c=512)
                            pv_ = pt[:].rearrange("p (h c) -> p h c", h=2, c=512)
                            nc.scalar.activation(pv_[:, :, coff:512], sv[:, :, coff:512],
                                                 AF.Exp, scale=0.125)
                            pend = (kt, pt)
                        emit_pv(*pend)
                        # normalize by the denominator row: reciprocals of the
                        # two heads' row 64 land in adjacent partitions of one
                        # tile, a single K=2 matmul with the 0/1 `sel` matrix
                        # broadcasts head A's to partitions 0:64 and head B's
                        # to 64:128, then two multiplies read straight out of
                        # the PSUM broadcast.
                        if phase < 3:
                            continue

                        def norm(qc=qc, g=g, otA=otA, otB=otB):
                            rc = rcp.tile([128, 512], F32R, tag="rc", name="rc")
                            nc.vector.reciprocal(rc[HD:HD + 1, :], otA[HD:HD + 1, :])
                            nc.vector.reciprocal(rc[32:33, :], otB[HD:HD + 1, :])
                            rb = mm_ps.tile([128, 512], F32, tag="mm", bufs=2, name="rb")
                            nc.tensor.matmul(rb[:, :], sel_s[HD:HD + 1, :],
                                             rc[HD:HD + 1, :], start=True, stop=False)
                            nc.tensor.matmul(rb[:, :], sel_s[32:33, :],
                                             rc[32:33, :], start=False, stop=True)
                            rcb = rcp.tile([128, 512], F32R, tag="rcb", name="rcb")
                            nc.vector.tensor_copy(rcb[:, :], rb[:, :])
                            for hp, ot in ((0, otA), (64, otB)):
                                nc.vector.tensor_tensor(
                                    otn_q[qc][hp:hp + 64, g * 512:(g + 1) * 512],
                                    ot[0:HD, :], rcb[hp:hp + 64, :], OP.mult)

                        pending_norm.append(norm)
                    # spend the reserved oproj pieces here: their matmuls give
                    # the PE cover while the g1 normalize reciprocals run
                    while fillers:
                        fillers.pop(0)()
                    flush_norm()



                # drive: interleave attention (and deferred O-proj) with the
                # QKV chunks -- attention for q-chunk sc needs only K/Q chunks
                # <= sc, so ACT's exp work overlaps the PE-dense projections.
                xta_next = qkv_dma(0)
                for sc in range(NSC):
                    qkv_chunk(sc, xta_next)
                    # prefetch the next chunk before the attention stretch so
                    # the SP queue streams it during compute
                    if sc + 1 < NSC:
                        xta_next = qkv_dma(sc + 1)
                    fillers = oproj_pieces(sc - 1) if (phase >= 4 and sc > 0) else []
                    if phase >= 2:
                        attention_qc(sc, fillers)
                    for f in fillers:
                        f()
                if phase >= 3:
                    flush_norm()
                if phase >= 4:
                    for f in oproj_pieces(NSC - 1, tail=True):
                        f()

            if use_loop:
                nb = 2 if repeat % 2 == 0 else 1
                with tc.For_i(0, repeat // nb, 1):
                    for _ in range(nb):
                        body()
            else:
                for _ in range(repeat):
                    body()

    _split_waits(nc)
    return nc


def _rope_tables(S):
    # interleaved dim order: within each 64-partition head block, partition
    # j=2i holds dim i (gets cos, -sin), j=2i+1 holds dim 32+i (cos, +sin)
    inv = 1.0 / (ROPE_BASE ** (np.arange(HALF, dtype=np.float64) / HALF))
    ang = np.arange(S, dtype=np.float64)[:, None] * inv[None, :]  # [S, HALF]
    cos, sin = np.cos(ang), np.sin(ang)
    j = np.arange(128) % HD
    freq = j // 2
    import ml_dtypes
    cs = cos[:, freq].T.astype(ml_dtypes.bfloat16)        # [128, S]
    sgn = np.where(j % 2 == 0, -1.0, 1.0)
    sn = (sin[:, freq] * sgn[None, :]).T.astype(ml_dtypes.bfloat16)
    return np.ascontiguousarray(cs), np.ascontiguousarray(sn)


def _tile_rows(a, nt):
    """[nt*128, C] -> [128, nt*C] with block kt at cols [kt*C, (kt+1)*C), bf16."""
    import ml_dtypes
    n, c = a.shape
    assert n == nt * 128
    t = a.reshape(nt, 128, c).transpose(1, 0, 2).reshape(128, nt * c)
    return np.ascontiguousarray(t.astype(ml_dtypes.bfloat16))


def _prep_x(x_b, D, S):
    """[S, D] -> [128, S*NKT] bf16: col sc*(NKT*512)+kt*512+s' = x[sc*512+s', kt*128+p]."""
    import ml_dtypes
    NKT, NSC = D // 128, S // 512
    t = x_b.reshape(NSC, 512, NKT, 128).transpose(3, 0, 2, 1).reshape(128, S * NKT)
    return np.ascontiguousarray(t.astype(ml_dtypes.bfloat16))


def _mask_consts():
    # additive causal mask in [k, q] layout: 0 where k <= q, else -400
    # (-50 after the 1/8 softmax scale -> exp underflows to ~2e-22).
    # bf16: both 0/-400 and 0/1 are exactly representable, and bf16 matmuls
    # run at 1 cycle/row even at free size 128 (fp32r pays 4x there).
    import ml_dtypes
    tri = np.where(np.triu(np.ones((128, 128), dtype=bool)), 0.0, -400.0).astype(ml_dtypes.bfloat16)
    idn = np.eye(128, dtype=ml_dtypes.bfloat16)
    return tri, idn


def _interleave_perm(n_heads):
    """Permutation of head-dim rows: new row 64h+2i <- old 64h+i,
    new row 64h+2i+1 <- old 64h+32+i."""
    perm = np.empty(n_heads * HD, dtype=np.int64)
    for h in range(n_heads):
        base = h * HD
        for i in range(HALF):
            perm[base + 2 * i] = base + i
            perm[base + 2 * i + 1] = base + HALF + i
    return perm


_PROG_CACHE = {}


def make_in_maps(x, Wq, Wk, Wv, Wo):
    B, S, D = x.shape
    H = 16
    HPC = 4                      # heads per core
    GROUPS = H // HPC            # 4 head-groups
    N_CORES = B * GROUPS

    x = np.asarray(x, dtype=np.float32)
    Wq, Wk, Wv, Wo = (np.asarray(w, dtype=np.float32) for w in (Wq, Wk, Wv, Wo))

    cs, sn = _rope_tables(S)
    tri, idn = _mask_consts()
    NKT = D // 128
    xTs = [_prep_x(x[b], D, S) for b in range(B)]

    perm = _interleave_perm(HPC)
    in_maps = []
    for c in range(N_CORES):
        b, hg = divmod(c, GROUPS)
        e0 = hg * HPC * HD
        e1 = e0 + HPC * HD
        in_maps.append({
            "xT": xTs[b],
            "wq": _tile_rows(Wq[e0:e1, :][perm].T, NKT),
            "wk": _tile_rows(Wk[e0:e1, :][perm].T, NKT),
            "wv": _tile_rows(Wv[e0:e1, :].T, NKT),
            "wo": _tile_rows(Wo[:, e0:e1].T, 2),
            "cs": cs, "sn": sn, "tri": tri, "idn": idn,
        })
    return in_maps


def kernel(x, Wq, Wk, Wv, Wo):
    B, S, D = x.shape
    H = 16
    HPC = 4                      # heads per core
    GROUPS = H // HPC            # 4 head-groups
    N_CORES = B * GROUPS

    in_maps = make_in_maps(x, Wq, Wk, Wv, Wo)

    key = (S, D, HPC)
    if key not in _PROG_CACHE:
        _PROG_CACHE[key] = build_program(S, D, HPC)
    nc = _PROG_CACHE[key]
    res = run_bass_kernel_spmd(nc, in_maps, list(range(N_CORES)))

    out = np.zeros((B, S, D), dtype=np.float64)
    for c in range(N_CORES):
        b = c // GROUPS
        out[b] += res.results[c]["out"].astype(np.float64)
    return out.astype(np.float32)


if __name__ == "__main__":
    # mini self-test: one core, small S/D, against a numpy model
    S, D, HPC = 512, 256, 4
    rng = np.random.default_rng(0)
    x = rng.standard_normal((S, D)).astype(np.float32)
    bound = 1.0 / np.sqrt(D)
    Wq, Wk, Wv = (rng.uniform(-bound, bound, (HPC * HD, D)).astype(np.float32) for _ in range(3))
    Wo = rng.uniform(-bound, bound, (D, HPC * HD)).astype(np.float32)

    # numpy reference (same math as reference.py, restricted to HPC heads)
    q = (x @ Wq.T).reshape(S, HPC, HD).transpose(1, 0, 2)
    k = (x @ Wk.T).reshape(S, HPC, HD).transpose(1, 0, 2)
    v = (x @ Wv.T).reshape(S, HPC, HD).transpose(1, 0, 2)
    inv = 1.0 / (ROPE_BASE ** (np.arange(HALF) / HALF))
    ang = np.arange(S)[:, None] * inv[None, :]
    cosr, sinr = np.cos(ang), np.sin(ang)

    def rope(t):
        t1, t2 = t[..., :HALF], t[..., HALF:]
        return np.concatenate([t1 * cosr - t2 * sinr, t1 * sinr + t2 * cosr], -1)

    q, k = rope(q), rope(k)
    sc_ = np.einsum("hqd,hkd->hqk", q, k) / np.sqrt(HD)
    mask = np.tril(np.ones((S, S), dtype=bool))
    sc_ = np.where(mask, sc_, -np.inf)
    p = np.exp(sc_ - sc_.max(-1, keepdims=True))
    p /= p.sum(-1, keepdims=True)
    ref = np.einsum("hqk,hkd->hqd", p, v).transpose(1, 0, 2).reshape(S, HPC * HD) @ Wo.T

    cs, sn = _rope_tables(S)
    tri, idn = _mask_consts()
    perm = _interleave_perm(HPC)
    in_map = {
        "xT": _prep_x(x, D, S),
        "wq": _tile_rows(Wq[perm].T, D // 128),
        "wk": _tile_rows(Wk[perm].T, D // 128),
        "wv": _tile_rows(Wv.T, D // 128),
        "wo": _tile_rows(Wo.T, 2),
        "cs": cs, "sn": sn, "tri": tri, "idn": idn,
    }
    nc = build_program(S, D, HPC)
    res = run_bass_kernel_spmd(nc, [in_map], [0])
    got = res.results[0]["out"]
    err = np.abs(got - ref)
    rel = err.max() / np.abs(ref).max()
    rms = np.sqrt((err ** 2).mean()) / np.sqrt((ref ** 2).mean())
    print(f"mini: max abs err {err.max():.3e}  max rel {rel:.3e}  rms rel {rms:.3e}")



# revision 3
# speedup vs baseline: 1.2064x; 1.2064x over previous
"""Trainium2 Bass kernel: multi-head causal self-attention with RoPE.

Computes, for x:[B,S,D], Wq/Wk/Wv/Wo:[D,D] (B=2, S=2048, D=1024, H=16 heads,
hd=64):
    q/k/v = (x @ W{q,k,v}.T) -> [B,H,S,hd];  q,k = rope(q), rope(k)
    out   = softmax(causal(q k^T / sqrt(hd))) v   -> merge heads -> @ Wo.T

Sharding: 8 NeuronCores = (2 batches) x (4 head-groups of 4 heads).  Each
core computes its 4 heads' attention plus the partial output projection
(columns of Wo belonging to its heads); the host sums the 4 partial outputs
per batch.

Per-core dataflow (everything in "transposed" space so no PE transposes are
needed):
    xT [D,S] -> QT,KT [hd,S] per head (fp32r matmuls) -> RoPE (DVE shuffle
    + mul/add) -> scoresT[k,q] = KT^T-slice matmuls -> exp on ACT (no
    max-subtraction: |scores/8| <= ~3.2) -> PV with a ones-column appended
    to V so row 64 of the accumulator is the softmax denominator ->
    normalize -> output projection from the transposed head outputs.

Pipelining notes (the tricks that matter for schedule overlap):
  - qtr/ktr/vv are PER-CHUNK tiles: Tile tracks deps per tile, so one big
    tile would false-serialize chunk sc+1's RoPE writes against chunk sc's
    attention reads and kill the cross-chunk pipeline.
  - PV accumulates both heads of a pair into ONE 2-bank PSUM tile (head A
    cols 0:512 / bank 0, head B cols 512:1024 / bank 1) which is evacuated
    to SBUF (bf16) immediately at group end; the softmax normalize
    (reciprocal of row 64, K=1 sel-matmul broadcast, two multiplies reading
    the broadcast straight from PSUM) then runs entirely off the critical
    path, so the next group's PV only waits for the evac.
  - x-chunk input DMAs and the output DMAs alternate between the two HWDGE
    rings (sync/SP and scalar/ACT); a single ring is FIFO and measures only
    ~80 GB/s on this access pattern.
"""

import sys

sys.path.insert(0, "/opt/trn_rl_repo")

import numpy as np

import concourse.bass as bass
import concourse.mybir as mybir
import concourse.tile as tile
from concourse.bass_utils import run_bass_kernel_spmd

F32 = mybir.dt.float32
F32R = mybir.dt.float32r
BF16 = mybir.dt.bfloat16
AF = mybir.ActivationFunctionType
OP = mybir.AluOpType

# stream_shuffle's 32-entry mask is a per-quadrant partition permutation
# (applied identically to all four 32-partition quadrants).  We therefore
# store head dims interleaved -- partition 64h+2i holds dim i, 64h+2i+1
# holds dim 32+i -- so the RoPE pair swap is an adjacent-pair exchange.
# The interleave is a shared permutation of Q and K dims (folded into the
# weight slices and rope tables on the host), which leaves q.k scores
# unchanged.
SWAP_MASK = [i ^ 1 for i in range(32)]

HD = 64
HALF = HD // 2
ROPE_BASE = 10000.0


def _split_waits(nc, maxw=1):
    """walrus in this container rejects instructions with more than a couple
    of semaphore waits; hoist excess waits onto preceding NoOps."""
    ctr = 0
    for bb in nc.main_func.blocks:
        insts = bb.instructions
        new = []
        changed = False
        for ins in insts:
            si = ins.sync_info
            if si is not None and si.on_wait and len(si.on_wait) > maxw:
                waits = list(si.on_wait)
                keep, rest = waits[:maxw], waits[maxw:]
                for i in range(0, len(rest), maxw):
                    ctr += 1
                    new.append(mybir.InstNoOp(
                        name=f"WSPLIT-{ctr}", opcode="NoOp", engine=ins.engine,
                        sync_info=mybir.SyncInfo(on_wait=rest[i:i + maxw], on_update=[])))
                si.on_wait = keep
                changed = True
            new.append(ins)
        if changed:
            bb.instructions = new


def build_program(S, D, HPC=4, repeat=1, use_loop=False, phase=4):
    """One-core SPMD program: attention for HPC heads of one batch.

    phase (for perf bisection): -1 = input DMA only, 0 = +QKV matmuls,
    1 = +rope/V-evac, 2 = +scores/exp/PV, 3 = +normalize, 4 = full.
    use_loop wraps `repeat` copies of the body in a tc.For_i (constant
    NEFF size -- used by the loop-slope timing harness).
    """
    NKT = D // 128          # k-tiles over the embedding dim
    NSC = S // 512          # 512-wide s-chunks
    NST = S // 128          # 128-wide s-tiles
    G = HPC // 2            # head pairs
    E = HPC * HD            # per-core head dims
    VW = HD + 1

    nc = bass.Bass()
    # pre-tiled host layouts: xT[p, sc*NKT*512 + kt*512 + s'] = x[sc*512+s', kt*128+p]
    xT = nc.declare_dram_parameter("xT", [128, S * NKT], BF16, isOutput=False)
    wq = nc.declare_dram_parameter("wq", [128, NKT * E], BF16, isOutput=False)
    wk = nc.declare_dram_parameter("wk", [128, NKT * E], BF16, isOutput=False)
    wv = nc.declare_dram_parameter("wv", [128, NKT * E], BF16, isOutput=False)
    wo = nc.declare_dram_parameter("wo", [128, G * D], BF16, isOutput=False)
    cs = nc.declare_dram_parameter("cs", [128, S], BF16, isOutput=False)
    sn = nc.declare_dram_parameter("sn", [128, S], BF16, isOutput=False)
    tri = nc.declare_dram_parameter("tri", [128, 128], BF16, isOutput=False)
    idn = nc.declare_dram_parameter("idn", [128, 128], BF16, isOutput=False)
    out = nc.declare_dram_parameter("out", [S, D], BF16, isOutput=True)

    with tile.TileContext(nc) as tc, \
         nc.allow_low_precision(reason="float32r operands feed the PE at full rate"):
        with (tc.tile_pool(name="wp", bufs=1) as wp,
              tc.tile_pool(name="xp", bufs=3) as xp,
              tc.tile_pool(name="rt", bufs=3) as rt,
              tc.tile_pool(name="ptp", bufs=4) as ptp,
              tc.tile_pool(name="rcp", bufs=3) as rcp,
              tc.tile_pool(name="osp", bufs=2) as osp,
              tc.tile_pool(name="oevp", bufs=3) as oevp,
              tc.tile_pool(name="ps", bufs=1, space="PSUM") as ps_pool,
              tc.tile_pool(name="ot_ps", bufs=1, space="PSUM") as ot_ps):
            qkv_ps = mm_ps = ps_pool  # shared PSUM pool; tags set per tile

            # ---- persistent tiles
            wq_s = wp.tile([128, NKT * E], BF16, name="wq_s")
            wk_s = wp.tile([128, NKT * E], BF16, name="wk_s")
            wv_s = wp.tile([128, NKT * E], BF16, name="wv_s")
            wo_s = wp.tile([128, G * D], BF16, name="wo_s")
            cs_s = wp.tile([128, S], BF16, name="cs_s")
            sn_s = wp.tile([128, S], BF16, name="sn_s")
            tri_s = wp.tile([128, 128], BF16, name="tri_s")
            sel_s = wp.tile([128, 256], BF16, name="sel_s")
            idn_s = wp.tile([128, 128], BF16, name="idn_s")
            # per-chunk Q^T/K^T and per-s-tile V tiles: separate tiles so a
            # later chunk's writes never false-depend on this chunk's reads
            qtr_c = [wp.tile([128, G * 512], BF16, name=f"qtr{i}") for i in range(NSC)]
            ktr_c = [wp.tile([128, G * 512], BF16, name=f"ktr{i}") for i in range(NSC)]
            vv_c = [wp.tile([128, HPC * VW], BF16, name=f"vv{i}") for i in range(NST)]
            # per-q-chunk otn tiles: oproj(qc-1) must not false-depend on
            # the (qc) normalize writes, which one big tile would impose
            otn_q = [wp.tile([128, G * 512], BF16, name=f"otn{i}")
                     for i in range(S // 512)]

            # weight/table loads issue from the ACT queue so the first x
            # chunk (SP queue) isn't stuck behind them; order = first use
            nc.scalar.dma_start(wq_s[:], wq[:])
            nc.scalar.dma_start(wk_s[:], wk[:])
            nc.scalar.dma_start(cs_s[:], cs[:])
            nc.scalar.dma_start(sn_s[:], sn[:])
            nc.scalar.dma_start(wv_s[:], wv[:])
            nc.scalar.dma_start(tri_s[:], tri[:])
            nc.scalar.dma_start(idn_s[:], idn[:])
            nc.scalar.dma_start(wo_s[:], wo[:])
            # ones columns of V+ (for the softmax denominator)
            for st in range(NST):
                vvr = vv_c[st][:].rearrange("p (h c) -> p h c", h=HPC, c=VW)
                nc.vector.memset(vvr[:, :, HD:HD + 1].bitcast(mybir.dt.uint16), 0x3f80)
            # sel: two K=1 row vectors at partition 64 for the normalize
            # broadcast -- cols 0:128 select out-partitions 0:64 (head A),
            # cols 128:256 select 64:128 (head B)
            nc.vector.memset(sel_s[64:65, :].bitcast(mybir.dt.uint16), 0)
            nc.vector.memset(sel_s[64:65, 0:HD].bitcast(mybir.dt.uint16), 0x3f80)
            nc.vector.memset(sel_s[64:65, 128 + HD:256].bitcast(mybir.dt.uint16), 0x3f80)

            def body():
                pending_norm = []
                # ================= QKV + RoPE =================
                def qkv_dma(sc):
                    xta = xp.tile([128, NKT * 512], BF16, tag="x", name="xta")
                    eng = nc.sync if sc % 2 == 0 else nc.scalar
                    eng.dma_start(xta[:], xT[:, sc * NKT * 512:(sc + 1) * NKT * 512])
                    return xta

                def qkv_chunk(sc, xta):
                    xts = [xta[:, kt * 512:(kt + 1) * 512] for kt in range(NKT)]

                    if phase == -1:
                        return
                    # Q^T and K^T, one head-pair (128 dims) at a time
                    for w_s, dst_c in ((wq_s, qtr_c), (wk_s, ktr_c)):
                        for g in range(G):
                            ps = qkv_ps.tile([128, 512], F32, tag="mm", bufs=2, name="ps")
                            for kt in range(NKT):
                                nc.tensor.matmul(
                                    ps[:], w_s[:, kt * E + g * 128: kt * E + (g + 1) * 128],
                                    xts[kt], start=(kt == 0), stop=(kt == NKT - 1))
                            if phase == 0:
                                continue
                            # RoPE: rot = ps*cos + swap(ps)*sgn_sin
                            qsw = rt.tile([128, 512], F32, tag="qsw", name="qsw")
                            m1 = rt.tile([128, 512], F32, tag="m1", name="m1")
                            m2 = rt.tile([128, 512], F32, tag="m2", name="m2")
                            # DVE does only the PSUM-reading shuffle + the m1
                            # product; the SBUF-only half of RoPE runs on the
                            # otherwise-idle Pool engine
                            nc.vector.stream_shuffle(qsw[:], ps[:], SWAP_MASK)
                            nc.vector.tensor_tensor(m1[:], ps[:], cs_s[:, sc * 512:(sc + 1) * 512], OP.mult)
                            nc.gpsimd.tensor_tensor(m2[:], qsw[:], sn_s[:, sc * 512:(sc + 1) * 512], OP.mult)
                            nc.gpsimd.tensor_tensor(
                                dst_c[sc][:, g * 512:(g + 1) * 512], m1[:], m2[:], OP.add)

                    # V (natural layout) for the 4 s-tiles of this chunk
                    for stl in range(4):
                        st = sc * 4 + stl
                        ps = qkv_ps.tile([128, 512], F32, tag="mm", bufs=2, name="psv")
                        for kt in range(NKT):
                            nc.tensor.matmul(
                                ps[:, 0:E], xta[:, kt * 512 + stl * 128: kt * 512 + (stl + 1) * 128],
                                wv_s[:, kt * E:(kt + 1) * E], start=(kt == 0), stop=(kt == NKT - 1))
                        if phase == 0:
                            continue
                        dst = vv_c[st][:].rearrange("p (h c) -> p h c", h=HPC, c=VW)[:, :, 0:HD]
                        nc.vector.tensor_copy(dst, ps[:, 0:E].rearrange("p (h c) -> p h c", h=HPC, c=HD))

                # ================= attention + output projection =================
                # Per (q-chunk, head-pair): the two heads' K=64 score matmuls
                # sit in different PE row groups (partition bases 0/64) and
                # overlap in the array.  k-tiles are processed two at a time
                # so one exp covers a [128,1024] two-bank PSUM tile.  The
                # causal mask is ADDED into the scores by an identity matmul
                # of a 0/-400 constant (exp then yields exact-enough zeros).

                def oproj_pieces(qc, tail=False):
                    """oproj(qc) as pieces of ~2 matmuls + a copy each,
                    for interleaving into the next q-chunk's attention loop
                    (PE cover for the exp hops; hides the mm->evac ping-pong)."""
                    CW = min(512, D)
                    oev_box = {}
                    pieces = []

                    def piece(st2, half, st, nch, on_act=False):
                        if half == 0 and nch == 0:
                            oev_box[st2] = oevp.tile([128, 2 * D], BF16, tag="oev", name="oev")
                        oev = oev_box[st2]
                        op = mm_ps.tile([128, 512], F32, tag="mm", bufs=2, name="opps")
                        for g in range(G):
                            nc.tensor.matmul(
                                op[:, 0:CW],
                                otn_q[qc][:, g * 512 + (st - qc * 4) * 128:
                                          g * 512 + (st - qc * 4 + 1) * 128],
                                wo_s[:, g * D + nch * CW: g * D + (nch + 1) * CW],
                                start=(g == 0), stop=(g == G - 1))
                        # in the tail block every exp is done, so ACT is idle:
                        # alternating the evac copies halves the serial drain
                        if on_act:
                            nc.scalar.copy(
                                oev[:, half * D + nch * CW: half * D + (nch + 1) * CW], op[:, 0:CW])
                        else:
                            nc.vector.tensor_copy(
                                oev[:, half * D + nch * CW: half * D + (nch + 1) * CW], op[:, 0:CW])
                        if half == 1 and nch == D // CW - 1:
                            st0 = qc * 4 + st2 * 2
                            dst = out[st0 * 128:(st0 + 2) * 128, :].rearrange(
                                "(b p) d -> p b d", b=2)
                            eng = nc.sync if (qc * 2 + st2) % 2 == 0 else nc.scalar
                            eng.dma_start(dst, oev[:].rearrange("p (b d) -> p b d", b=2))

                    idx = 0
                    for st2 in range(2):
                        for half in range(2):
                            for nch in range(D // CW):
                                on_act = tail and idx % 2 == 1
                                pieces.append(
                                    lambda st2=st2, half=half, nch=nch, on_act=on_act:
                                    piece(st2, half, qc * 4 + st2 * 2 + half, nch, on_act))
                                idx += 1
                    return pieces

                def flush_norm():
                    while pending_norm:
                        pending_norm.pop(0)()

                def attention_qc(qc, fillers=()):
                    fillers = list(fillers)
                    osb = osp.tile([128, 4 * 512], BF16, tag="osb", name="osb")
                    for g in range(G):
                        h0, h1 = 2 * g, 2 * g + 1
                        # both heads of the pair accumulate into ONE 2-bank
                        # PSUM tile: head A cols 0:512 (bank 0), head B cols
                        # 512:1024 (bank 1)
                        ot2 = ot_ps.tile([128, 1024], F32, tag="ot", name="ot2")
                        nkt_q = 4 * qc + 4

                        def emit_pv(kt, pt):
                            coff = max(0, 128 * kt - 512 * qc)
                            for hh, cb in ((h0, 0), (h1, 512)):
                                nc.tensor.matmul(
                                    ot2[0:VW, cb + coff:cb + 512],
                                    vv_c[kt][:, hh * VW:(hh + 1) * VW],
                                    pt[:, cb + coff:cb + 512],
                                    start=(kt == 0), stop=(kt == nkt_q - 1))

                        # software-pipelined: PV(kt-1) is emitted between the
                        # scores matmuls of kt and the exp of kt, so the PE
                        # never sits waiting on the exp it just enabled.  Both
                        # heads' scores live in one 2-bank PSUM tile (A at
                        # cols 0:512, B at 512:1024) so a single exp with a
                        # [p, 2, 512-coff] access pattern covers the pair.
                        pend = None
                        for kt in range(nkt_q):
                            coff = max(0, 128 * kt - 512 * qc)
                            diag = kt >= 4 * qc
                            ktc, ktl = kt // 4, kt % 4
                            s2 = mm_ps.tile([128, 1024], F32, tag="sc", bufs=2, name="s2")
                            for hp, cb in ((0, 0), (64, 512)):
                                nc.tensor.matmul(
                                    s2[:, cb + coff:cb + 512],
                                    ktr_c[ktc][hp:hp + 64, g * 512 + ktl * 128: g * 512 + (ktl + 1) * 128],
                                    qtr_c[qc][hp:hp + 64, g * 512 + coff: g * 512 + 512],
                                    start=True, stop=not diag)
                                if diag:
                                    nc.tensor.matmul(
                                        s2[:, cb + coff:cb + coff + 128],
                                        idn_s[:], tri_s[:], start=False, stop=True)
                            if kt == 0:
                                flush_norm()
                            if pend is not None:
                                emit_pv(*pend)
                            if len(fillers) > 3:
                                fillers.pop(0)()
                            pt = ptp.tile([128, 1024], BF16, tag="pt", bufs=4, name="pt")
                            sv = s2[:].rearrange("p (h c) -> p h c", h=2, c=512)
                            pv_ = pt[:].rearrange("p (h c) -> p h c", h=2, c=512)
                            nc.scalar.activation(pv_[:, :, coff:512], sv[:, :, coff:512],
                                                 AF.Exp, scale=0.125)
                            pend = (kt, pt)
                        emit_pv(*pend)
                        if phase < 3:
                            continue

                        # evacuate the pair to SBUF right away (bf16),
                        # releasing the PSUM banks; the normalize below then
                        # runs entirely from SBUF, off the critical path
                        for h in range(2):
                            nc.vector.tensor_copy(
                                osb[0:VW, (2 * g + h) * 512:(2 * g + h + 1) * 512],
                                ot2[0:VW, h * 512:(h + 1) * 512])

                        # normalize: reciprocal of the denominator row, then
                        # a K=1 matmul of the 0/1 `sel` rows broadcasts head
                        # A's reciprocals to partitions 0:64 and head B's to
                        # 64:128; the two multiplies read the broadcast
                        # straight out of PSUM (other operand is SBUF).
                        def norm(qc=qc, g=g, osb=osb):
                            rc = rcp.tile([128, 1024], BF16, tag="rc", name="rc")
                            nc.vector.reciprocal(rc[64:65, :], osb[64:65, g * 1024:(g + 1) * 1024])
                            rb = mm_ps.tile([128, 512], F32, tag="mm", bufs=2, name="rb")
                            nc.tensor.matmul(rb[:, :], sel_s[64:65, 0:128],
                                             rc[64:65, 0:512], start=True, stop=False)
                            nc.tensor.matmul(rb[:, :], sel_s[64:65, 128:256],
                                             rc[64:65, 512:1024], start=False, stop=True)
                            for h, hp in ((0, 0), (1, 64)):
                                nc.vector.tensor_tensor(
                                    otn_q[qc][hp:hp + 64, g * 512:(g + 1) * 512],
                                    osb[0:HD, (2 * g + h) * 512:(2 * g + h) * 512 + 512],
                                    rb[hp:hp + 64, :], OP.mult)

                        pending_norm.append(norm)
                    # spend the reserved oproj pieces here: their matmuls give
                    # the PE cover while the g1 normalize reciprocals run
                    while fillers:
                        fillers.pop(0)()
                    flush_norm()

                # drive: interleave attention (and deferred O-proj) with the
                # QKV chunks -- attention for q-chunk sc needs only K/Q chunks
                # <= sc, so ACT's exp work overlaps the PE-dense projections.
                xta_next = qkv_dma(0)
                for sc in range(NSC):
                    qkv_chunk(sc, xta_next)
                    # prefetch the next chunk before the attention stretch so
                    # the other HWDGE ring streams it during compute
                    if sc + 1 < NSC:
                        xta_next = qkv_dma(sc + 1)
                    fillers = oproj_pieces(sc - 1) if (phase >= 4 and sc > 0) else []
                    if phase >= 2:
                        attention_qc(sc, fillers)
                    for f in fillers:
                        f()
                if phase >= 3:
                    flush_norm()
                if phase >= 4:
                    for f in oproj_pieces(NSC - 1, tail=True):
                        f()

            if use_loop:
                nb = 2 if repeat % 2 == 0 else 1
                with tc.For_i(0, repeat // nb, 1):
                    for _ in range(nb):
                        body()
            else:
                for _ in range(repeat):
                    body()

    _split_waits(nc)
    return nc


def _rope_tables(S):
    # interleaved dim order: within each 64-partition head block, partition
    # j=2i holds dim i (gets cos, -sin), j=2i+1 holds dim 32+i (cos, +sin)
    inv = 1.0 / (ROPE_BASE ** (np.arange(HALF, dtype=np.float64) / HALF))
    ang = np.arange(S, dtype=np.float64)[:, None] * inv[None, :]  # [S, HALF]
    cos, sin = np.cos(ang), np.sin(ang)
    j = np.arange(128) % HD
    freq = j // 2
    import ml_dtypes
    cs = cos[:, freq].T.astype(ml_dtypes.bfloat16)        # [128, S]
    sgn = np.where(j % 2 == 0, -1.0, 1.0)
    sn = (sin[:, freq] * sgn[None, :]).T.astype(ml_dtypes.bfloat16)
    return np.ascontiguousarray(cs), np.ascontiguousarray(sn)


def _tile_rows(a, nt):
    """[nt*128, C] -> [128, nt*C] with block kt at cols [kt*C, (kt+1)*C), bf16."""
    import ml_dtypes
    n, c = a.shape
    assert n == nt * 128
    t = a.reshape(nt, 128, c).transpose(1, 0, 2).reshape(128, nt * c)
    return np.ascontiguousarray(t.astype(ml_dtypes.bfloat16))


def _prep_x(x_b, D, S):
    """[S, D] -> [128, S*NKT] bf16: col sc*(NKT*512)+kt*512+s' = x[sc*512+s', kt*128+p]."""
    import ml_dtypes
    NKT, NSC = D // 128, S // 512
    t = x_b.reshape(NSC, 512, NKT, 128).transpose(3, 0, 2, 1).reshape(128, S * NKT)
    return np.ascontiguousarray(t.astype(ml_dtypes.bfloat16))


def _mask_consts():
    # additive causal mask in [k, q] layout: 0 where k <= q, else -400
    # (-50 after the 1/8 softmax scale -> exp underflows to ~2e-22).
    # bf16: both 0/-400 and 0/1 are exactly representable, and bf16 matmuls
    # run at 1 cycle/row even at free size 128 (fp32r pays 4x there).
    import ml_dtypes
    tri = np.where(np.triu(np.ones((128, 128), dtype=bool)), 0.0, -400.0).astype(ml_dtypes.bfloat16)
    idn = np.eye(128, dtype=ml_dtypes.bfloat16)
    return tri, idn


def _interleave_perm(n_heads):
    """Permutation of head-dim rows: new row 64h+2i <- old 64h+i,
    new row 64h+2i+1 <- old 64h+32+i."""
    perm = np.empty(n_heads * HD, dtype=np.int64)
    for h in range(n_heads):
        base = h * HD
        for i in range(HALF):
            perm[base + 2 * i] = base + i
            perm[base + 2 * i + 1] = base + HALF + i
    return perm


_PROG_CACHE = {}


def make_in_maps(x, Wq, Wk, Wv, Wo):
    B, S, D = x.shape
    H = 16
    HPC = 4                      # heads per core
    GROUPS = H // HPC            # 4 head-groups
    N_CORES = B * GROUPS

    x = np.asarray(x, dtype=np.float32)
    Wq, Wk, Wv, Wo = (np.asarray(w, dtype=np.float32) for w in (Wq, Wk, Wv, Wo))

    cs, sn = _rope_tables(S)
    tri, idn = _mask_consts()
    NKT = D // 128
    xTs = [_prep_x(x[b], D, S) for b in range(B)]

    perm = _interleave_perm(HPC)
    in_maps = []
    for c in range(N_CORES):
        b, hg = divmod(c, GROUPS)
        e0 = hg * HPC * HD
        e1 = e0 + HPC * HD
        in_maps.append({
            "xT": xTs[b],
            "wq": _tile_rows(Wq[e0:e1, :][perm].T, NKT),
            "wk": _tile_rows(Wk[e0:e1, :][perm].T, NKT),
            "wv": _tile_rows(Wv[e0:e1, :].T, NKT),
            "wo": _tile_rows(Wo[:, e0:e1].T, 2),
            "cs": cs, "sn": sn, "tri": tri, "idn": idn,
        })
    return in_maps


def kernel(x, Wq, Wk, Wv, Wo):
    B, S, D = x.shape
    H = 16
    HPC = 4                      # heads per core
    GROUPS = H // HPC            # 4 head-groups
    N_CORES = B * GROUPS

    in_maps = make_in_maps(x, Wq, Wk, Wv, Wo)

    key = (S, D, HPC)
    if key not in _PROG_CACHE:
        _PROG_CACHE[key] = build_program(S, D, HPC)
    nc = _PROG_CACHE[key]
    res = run_bass_kernel_spmd(nc, in_maps, list(range(N_CORES)))

    out = np.zeros((B, S, D), dtype=np.float64)
    for c in range(N_CORES):
        b = c // GROUPS
        out[b] += res.results[c]["out"].astype(np.float64)
    return out.astype(np.float32)


if __name__ == "__main__":
    # mini self-test: one core, small S/D, against a numpy model
    S, D, HPC = 512, 256, 4
    rng = np.random.default_rng(0)
    x = rng.standard_normal((S, D)).astype(np.float32)
    bound = 1.0 / np.sqrt(D)
    Wq, Wk, Wv = (rng.uniform(-bound, bound, (HPC * HD, D)).astype(np.float32) for _ in range(3))
    Wo = rng.uniform(-bound, bound, (D, HPC * HD)).astype(np.float32)

    # numpy reference (same math as reference.py, restricted to HPC heads)
    q = (x @ Wq.T).reshape(S, HPC, HD).transpose(1, 0, 2)
    k = (x @ Wk.T).reshape(S, HPC, HD).transpose(1, 0, 2)
    v = (x @ Wv.T).reshape(S, HPC, HD).transpose(1, 0, 2)
    inv = 1.0 / (ROPE_BASE ** (np.arange(HALF) / HALF))
    ang = np.arange(S)[:, None] * inv[None, :]
    cosr, sinr = np.cos(ang), np.sin(ang)

    def rope(t):
        t1, t2 = t[..., :HALF], t[..., HALF:]
        return np.concatenate([t1 * cosr - t2 * sinr, t1 * sinr + t2 * cosr], -1)

    q, k = rope(q), rope(k)
    sc_ = np.einsum("hqd,hkd->hqk", q, k) / np.sqrt(HD)
    mask = np.tril(np.ones((S, S), dtype=bool))
    sc_ = np.where(mask, sc_, -np.inf)
    p = np.exp(sc_ - sc_.max(-1, keepdims=True))
    p /= p.sum(-1, keepdims=True)
    ref = np.einsum("hqk,hkd->hqd", p, v).transpose(1, 0, 2).reshape(S, HPC * HD) @ Wo.T

    cs, sn = _rope_tables(S)
    tri, idn = _mask_consts()
    perm = _interleave_perm(HPC)
    in_map = {
        "xT": _prep_x(x, D, S),
        "wq": _tile_rows(Wq[perm].T, D // 128),
        "wk": _tile_rows(Wk[perm].T, D // 128),
        "wv": _tile_rows(Wv.T, D // 128),
        "wo": _tile_rows(Wo.T, 2),
        "cs": cs, "sn": sn, "tri": tri, "idn": idn,
    }
    nc = build_program(S, D, HPC)
    res = run_bass_kernel_spmd(nc, [in_map], [0])
    got = res.results[0]["out"]
    err = np.abs(got - ref)
    rel = err.max() / np.abs(ref).max()
    rms = np.sqrt((err ** 2).mean()) / np.sqrt((ref ** 2).mean())
    print(f"mini: max abs err {err.max():.3e}  max rel {rel:.3e}  rms rel {rms:.3e}")
